# revision 3
# baseline (speedup 1.0000x reference)
"""Trainium2 Bass kernel for MultiHeadSelfAttention (RoPE + causal softmax).

Problem (hardcoded):
  x: (2, 2048, 512) f32, w_qkv: (1536, 512) f32, w_o: (512, 512) f32
  D_MODEL=512, N_HEADS=16, HEAD_DIM=32, ROPE_BASE=10000, causal.

Sharding: tensor-parallel over heads. Core c owns heads (2c, 2c+1) for both
batches. Each core computes its heads' q/k/v projections from the full x,
attention, and a Megatron-style row-parallel partial of the output
projection (out @ w_o.T restricted to its 64 input features). The host sums
the 8 partials (the row-parallel "unshard").

On-core layout highlights:
  - x is passed pre-transposed (xT [512, 4096], bf16) so the d_model
    contraction sits on the partition axis for both projection orientations.
  - q,k are produced transposed ([feat, row]) and RoPE'd in that layout
    (rotate_half done with a block-diag permutation matmul on the PE).
  - scores are computed transposed (S.T [keys, queries]) so softmax'd P
    feeds the av matmul as weights without any transpose.
  - causal mask applied by accumulating -240*max(0, k-q) into the scores
    PSUM via a rank-128 A.T@B matmul of constant triangular matrices.
  - exp on the scalar engine (PSUM -> bf16 SBUF), with 1/sqrt(hd) folded
    into the activation scale. No max-subtraction (scores are provably
    small for this problem's scale).
  - row sums for softmax come from an extra all-ones column appended to v.
  - weights/activations in bf16 (halves HBM traffic); PSUM accum is f32.
  - output partials DMA'd in bf16; the host reduces in f32.
  - xt double-buffered with explicit prefetch so the next batch/iteration's
    x reads overlap attention instead of waiting at the phase boundary.
"""

import sys
import math
from contextlib import ExitStack

sys.path.insert(0, "/opt/trn_rl_repo")

import numpy as np
import ml_dtypes

import concourse.bass as bass
import concourse.tile as tile
from concourse import bacc, mybir
from concourse.bass_utils import run_bass_kernel_spmd

F32 = mybir.dt.float32
F32R = mybir.dt.float32r
BF16 = mybir.dt.bfloat16
EXP = mybir.ActivationFunctionType.Exp

B = 2
T = 2048
D = 512
NH = 16
HD = 32
NCORES = 8
R = B * T            # 4096 rows, row = b*T + t
NHL = NH // NCORES   # 2 heads per core
KC = T // 128        # 16 key chunks per batch
SCALE = 1.0 / math.sqrt(HD)
MASK_VAL = -240.0

LOWP = True          # bf16 x / w_qkv / cos / sin / q / k path
XDT = BF16 if LOWP else F32R
NPX = ml_dtypes.bfloat16 if LOWP else np.float32


def _bcast_free(ap_2d, n_inner):
    """[P, n] -> [P, n, n_inner] AP with the inner dim broadcast (step 0)."""
    return bass.AP(
        tensor=ap_2d.tensor,
        offset=ap_2d.offset,
        ap=list(ap_2d.ap[:-1]) + [list(ap_2d.ap[-1]), [0, n_inner]],
    )


def _emit(tc, io, loop_k=1):
    nc = tc.nc
    with ExitStack() as ctx:
        cpool = ctx.enter_context(tc.tile_pool(name="consts", bufs=1))
        mpool = ctx.enter_context(tc.tile_pool(name="main", bufs=1))
        spool = ctx.enter_context(tc.tile_pool(name="small", bufs=3))
        ppool = ctx.enter_context(tc.tile_pool(name="pk", bufs=2))
        # single PSUM pool, tags shared across phases (8 banks total):
        #   tagA [128,1024] x2 = 4 banks  (qk-proj / scores)
        #   tagB [128,512]  x2 = 2 banks  (shift/vT/vtr/atps/outps)
        #   tagC [128,8,33] x2 = 2 banks  (av accumulator groups)
        psum = ctx.enter_context(tc.tile_pool(name="psum", bufs=1, space="PSUM"))

        def tile_a():
            return psum.tile([128, 1024], F32, tag="A", bufs=2, name="psA")

        def tile_b(p=128, w=512):
            return psum.tile([p, w], F32, tag="B", bufs=2, name="psB")

        def tile_c():
            return psum.tile([128, 8, HD + 1], F32, tag="C", bufs=2, name="psC")

        # ---- constants (one-time DMAs, spread over issue queues) ----
        wo = cpool.tile([64, 512], BF16, tag="wo")
        nc.scalar.dma_start(out=wo, in_=io["woT"])
        cmix = cpool.tile([128, 384], BF16, tag="cmix")
        nc.gpsimd.dma_start(out=cmix, in_=io["consts1"])
        permt = cmix[:, 0:128]
        trilA = cmix[:, 128:256]
        trilB = cmix[:, 256:384]
        identb = cpool.tile([128, 128], BF16, tag="identb")
        nc.gpsimd.dma_start(out=identb, in_=io["ident"])
        wqkv = []
        for dc in range(4):
            w_t = cpool.tile([128, 192], XDT, tag=f"wqkv{dc}")
            nc.gpsimd.dma_start(out=w_t, in_=io["wqkvT"][dc * 128:(dc + 1) * 128, :])
            wqkv.append(w_t)
        cosw = mpool.tile([128, T], XDT, tag="cosw")        # one batch (shared)
        sinw = mpool.tile([128, T], XDT, tag="sinw")
        nc.scalar.dma_start(out=cosw, in_=io["cosw"])
        nc.scalar.dma_start(out=sinw, in_=io["sinw"])

        # ---- persistent activations ----
        qkr = mpool.tile([128, R], XDT, tag="qkr")          # RoPE'd qT/kT
        ka = mpool.tile([64, R], XDT, tag="ka")             # k-half, base-aligned
        vall = mpool.tile([128, R // 128, NHL, HD + 1], BF16, tag="vall")
        ao = mpool.tile([128, B, KC, NHL, HD], BF16, tag="ao")  # attnout natural
        aoT = mpool.tile([64, R], BF16, tag="aoT")          # attnout transposed

        nc.vector.memset(vall[:, :, :, HD:HD + 1], 1.0)     # softmax-sum column

        def prefetch_xt(bb):
            xt = [mpool.tile([128, T], XDT, tag=f"xt{dc}", bufs=2,
                             name=f"xt{dc}") for dc in range(4)]
            for j in range(4):
                for dc in range(4):
                    nc.sync.dma_start(
                        out=xt[dc][:, j * 512:(j + 1) * 512],
                        in_=io["xT"][dc * 128:(dc + 1) * 128,
                                     bb * T + j * 512:bb * T + (j + 1) * 512],
                    )
            return xt

        def emit_proj(bb, xt):
            for jl in range(4):
                colb = slice(jl * 512, (jl + 1) * 512)          # batch-local
                cols = slice(bb * T + jl * 512, bb * T + (jl + 1) * 512)
                # qT/kT projection: [feat, row] = wqkT.T @ xT
                qk_ps = tile_b()
                for dc in range(4):
                    nc.tensor.matmul(
                        qk_ps, wqkv[dc][:, 0:128], xt[dc][:, colb],
                        start=(dc == 0), stop=(dc == 3),
                    )
                # rotate_half via block-diag permutation (needs SBUF copy);
                # the copies ride the scalar engine (ACT has HW headroom)
                qks = spool.tile([128, 512], BF16, tag="qks")
                nc.scalar.copy(qks, qk_ps)
                sh_ps = tile_b()
                nc.tensor.matmul(sh_ps, permt, qks, start=True,
                                 stop=True)
                sh_sb = spool.tile([128, 512], BF16, tag="shsb")
                nc.scalar.copy(sh_sb, sh_ps)
                # qkr = qk*cos + shifted*sin_signed on the gpsimd engine
                # (all-SBUF operands; frees the DVE), rounding to bf16 only
                # at the final add
                t1 = spool.tile([128, 512], F32, tag="t1")
                t2 = spool.tile([128, 512], F32, tag="t2")
                nc.gpsimd.tensor_mul(t1, sh_sb, sinw[:, colb])
                nc.gpsimd.tensor_mul(t2, qks, cosw[:, colb])
                nc.gpsimd.tensor_add(qkr[:, cols], t2, t1)
                # partition-aligned copy of the k rows (matmul requires lhsT
                # and rhs to share a base partition)
                nc.vector.tensor_copy(ka[:, cols], qkr[64:128, cols])

                # vT projection: [feat, row]
                vt_ps = tile_b(64)
                for dc in range(4):
                    nc.tensor.matmul(
                        vt_ps, wqkv[dc][:, 128:192], xt[dc][:, colb],
                        start=(dc == 0), stop=(dc == 3),
                    )
                vt_sb = spool.tile([64, 512], BF16, tag="vtsb")
                nc.scalar.copy(vt_sb, vt_ps)
                # transpose v back to natural [row, feat] (bf16 on copy-out);
                # 4 transposes share one PSUM bank (disjoint 64-col regions)
                vtr_ps = psum.tile([128, 256], BF16, tag="B", bufs=2,
                                   name="psBv")
                for jj in range(4):
                    nc.tensor.transpose(
                        vtr_ps[:, jj * 64:(jj + 1) * 64],
                        vt_sb[:, jj * 128:(jj + 1) * 128],
                        identb[0:64, 0:64],
                    )
                for jj in range(4):
                    nc.vector.tensor_copy(
                        vall[:, bb * KC + jl * 4 + jj, :, 0:HD],
                        vtr_ps[:, jj * 64:(jj + 1) * 64])

        def emit_attention(bb, hh):
            if True:
                qrow = 32 * hh            # q rows in qkr
                krow = 32 * hh            # k rows in ka
                ppks = []
                pavs = {}

                def av_column(qc):
                    # av column for qc (P rows kc<=qc all exist);
                    # 8 query chunks per PSUM bank, normalized per group
                    g = qc // 8
                    if qc % 8 == 0:
                        pavs[g] = tile_c()
                    slot = pavs[g][:, qc % 8, :]
                    for kp in range(qc + 1):
                        nc.tensor.matmul(
                            slot,
                            ppks[kp][:, 128 * (qc - kp):128 * (qc - kp) + 128],
                            vall[:, bb * KC + kp, hh, :],
                            start=(kp == 0), stop=(kp == qc),
                        )
                    if qc % 8 == 7:
                        # normalize this group: attnout = av / l
                        pav = pavs[g]
                        rl = spool.tile([128, 8, 1], F32, tag="rl")
                        nc.vector.reciprocal(rl, pav[:, :, HD:HD + 1])
                        nc.vector.tensor_mul(
                            ao[:, bb, g * 8:(g + 1) * 8, hh, :],
                            pav[:, :, 0:HD],
                            _bcast_free(rl[:, :, 0], HD),
                        )

                # av columns trail the score/exp stream by 2 key chunks so
                # the PE never stalls waiting for the exp it just queued
                for kc in range(KC + 2):
                    if kc < KC:
                        n_kc = T - 128 * kc
                        cw = 1024
                        kslc = slice(bb * T + 128 * kc, bb * T + 128 * (kc + 1))
                        # per-kc P tile: precise deps (av reads never block
                        # later exps) and half the packed-tile footprint
                        ppk = ppool.tile([128, n_kc], BF16, tag=f"ppk{kc}",
                                         bufs=(2 if kc < 5 else 1),
                                         name=f"ppk{kc}")
                        ppks.append(ppk)
                        for c0 in range(0, n_kc, cw):
                            nt = min(cw, n_kc - c0)
                            sc_ps = tile_a()
                            for c in range(c0, c0 + nt, 512):
                                ln = min(512, n_kc - c)
                                qslc = slice(bb * T + 128 * kc + c,
                                             bb * T + 128 * kc + c + ln)
                                nc.tensor.matmul(
                                    sc_ps[:, c - c0:c - c0 + ln],
                                    ka[krow:krow + 32, kslc],
                                    qkr[qrow:qrow + 32, qslc],
                                    start=True, stop=(c > 0),
                                    skip_group_check=True,
                                )
                            if c0 == 0:
                                # causal mask on the diagonal 128x128 block:
                                # accumulates -240*max(0, k-q)
                                nc.tensor.matmul(
                                    sc_ps[:, 0:128], trilA, trilB,
                                    start=False, stop=True,
                                    skip_group_check=True,
                                )
                            nc.scalar.activation(
                                out=ppk[:, c0:c0 + nt],
                                in_=sc_ps[:, 0:nt],
                                func=EXP, scale=SCALE,
                            )
                    if kc >= 2:
                        av_column(kc - 2)

        def emit_epilogue(bb, last):
            # transpose attnout group g, then immediately out-proj its 4
            # row chunks so the tail drains incrementally
            for g in range(4):
                at_ps = psum.tile([64, 512], BF16, tag="B", bufs=2,
                                  name="psBt")
                for jj in range(4):
                    qc = g * 4 + jj
                    nc.tensor.transpose(
                        at_ps[:, jj * 128:(jj + 1) * 128],
                        ao[:, bb, qc, :, :].rearrange("p a b -> p (a b)"),
                        identb,
                    )
                if last and g % 2 == 1:
                    nc.scalar.copy(
                        aoT[:, bb * T + g * 512:bb * T + (g + 1) * 512],
                        at_ps)
                else:
                    nc.vector.tensor_copy(
                        aoT[:, bb * T + g * 512:bb * T + (g + 1) * 512],
                        at_ps)
                for qc in range(g * 4, g * 4 + 4):
                    rc = bb * KC + qc
                    out_ps = tile_b()
                    nc.tensor.matmul(
                        out_ps, aoT[:, rc * 128:(rc + 1) * 128],
                        wo, start=True, stop=True,
                    )
                    out_sb = spool.tile([128, 512], BF16, tag="outsb", bufs=8)
                    nc.scalar.copy(out_sb, out_ps)
                    eng = nc.sync if qc % 2 == 0 else nc.gpsimd
                    eng.dma_start(
                        out=io["out_part"][rc * 128:(rc + 1) * 128, :],
                        in_=out_sb,
                    )

        # software-pipelined emission: later batches' proj and earlier
        # batches' epilogues fill engine gaps in the exp-paced attention;
        # xt prefetches overlap the next batch's x reads with attention
        xt0 = prefetch_xt(0)
        for _it in range(loop_k):
            xt1 = prefetch_xt(1)
            emit_proj(0, xt0)
            emit_attention(0, 0)
            xt0n = prefetch_xt(0) if _it < loop_k - 1 else None
            emit_attention(0, 1)
            emit_proj(1, xt1)
            emit_attention(1, 0)
            emit_epilogue(0, last=False)
            emit_attention(1, 1)
            emit_epilogue(1, last=True)
            xt0 = xt0n


def build_program(loop_k=1):
    nc = bacc.Bacc(
        "TRN2", target_bir_lowering=False, debug=False,
        enable_asserts=True, num_devices=NCORES,
    )
    io = {}
    for name, shape, dt_ in [
        ("xT", [D, R], XDT), ("wqkvT", [D, 192], XDT),
        ("woT", [64, D], BF16),
        ("cosw", [128, T], XDT), ("sinw", [128, T], XDT),
        ("consts1", [128, 384], BF16), ("ident", [128, 128], BF16),
    ]:
        io[name] = nc.dram_tensor(name, shape, dt_, kind="ExternalInput").ap()
    io["out_part"] = nc.dram_tensor("out_part", [R, D], BF16,
                                    kind="ExternalOutput").ap()
    with tile.TileContext(nc) as tc:
        _emit(tc, io, loop_k=loop_k)
    nc.compile()
    return nc


def host_constants():
    t = np.arange(T, dtype=np.float32)
    inv_freq = (1.0 / (10000.0 ** (np.arange(0, HD, 2, dtype=np.float32) / HD)))
    freqs = np.outer(t, inv_freq).astype(np.float32)      # (T, 16)
    emb = np.concatenate([freqs, freqs], axis=-1)         # (T, 32)
    cos = np.cos(emb).astype(np.float32)
    sin = np.sin(emb).astype(np.float32)
    cosw = np.tile(cos.T, (4, 1)).astype(NPX)             # (128, 2048)
    ssin = sin.T.copy()
    ssin[:HD // 2] *= -1.0                                # signed sin
    sinw = np.tile(ssin, (4, 1)).astype(NPX)

    permt = np.zeros((128, 128), dtype=np.float32)
    for blk in range(4):
        for m in range(HD):
            permt[blk * HD + (m + HD // 2) % HD, blk * HD + m] = 1.0

    a = np.arange(128)
    trilA = np.where(a[:, None] <= a[None, :], MASK_VAL, 0.0).astype(np.float32)
    trilB = np.where(a[:, None] > a[None, :], 1.0, 0.0).astype(np.float32)
    ident = np.eye(128, dtype=np.float32).astype(ml_dtypes.bfloat16)
    consts1 = np.concatenate([permt, trilA, trilB], axis=1).astype(ml_dtypes.bfloat16)
    return dict(cosw=cosw, sinw=sinw, ident=ident,
                consts1=np.ascontiguousarray(consts1))


def core_inputs(x, w_qkv, w_o):
    """Per-core input maps (core c owns heads 2c, 2c+1)."""
    x = np.asarray(x, dtype=np.float32)
    w_qkv = np.asarray(w_qkv, dtype=np.float32)
    w_o = np.asarray(w_o, dtype=np.float32)
    xT = np.ascontiguousarray(x.reshape(R, D).T).astype(NPX)
    consts = host_constants()
    maps = []
    for c in range(NCORES):
        h0 = NHL * c
        qrows = w_qkv[h0 * HD:(h0 + NHL) * HD]                  # (64, 512)
        krows = w_qkv[D + h0 * HD:D + (h0 + NHL) * HD]
        vrows = w_qkv[2 * D + h0 * HD:2 * D + (h0 + NHL) * HD]
        m = dict(consts)
        m["xT"] = xT
        m["wqkvT"] = np.ascontiguousarray(
            np.concatenate([qrows, krows, vrows], axis=0).T).astype(NPX)
        m["woT"] = np.ascontiguousarray(
            w_o[:, h0 * HD:(h0 + NHL) * HD].T).astype(ml_dtypes.bfloat16)
        maps.append(m)
    return maps


_PROG = None


def _get_prog():
    global _PROG
    if _PROG is None:
        _PROG = build_program()
    return _PROG


def kernel(x, w_qkv, w_o):
    nc = _get_prog()
    maps = core_inputs(x, w_qkv, w_o)
    res = run_bass_kernel_spmd(nc, maps, list(range(NCORES)))
    acc = np.zeros((R, D), dtype=np.float32)
    for i in range(NCORES):
        acc += res.results[i]["out_part"].astype(np.float32)
    return acc.reshape(B, T, D)


# revision 4
# speedup vs baseline: 1.3020x; 1.3020x over previous
"""Trainium2 Bass kernel for MultiHeadSelfAttention (RoPE + causal softmax).

Problem (hardcoded):
  x: (2, 2048, 512) f32, w_qkv: (1536, 512) f32, w_o: (512, 512) f32
  D_MODEL=512, N_HEADS=16, HEAD_DIM=32, ROPE_BASE=10000, causal.

Sharding: tensor-parallel over heads. Core c owns heads (2c, 2c+1) for both
batches. Each core computes its heads' q/k/v projections from the full x,
attention, and a Megatron-style row-parallel partial of the output
projection (out @ w_o.T restricted to its 64 input features). The host sums
the 8 partials (the row-parallel "unshard").

On-core layout highlights:
  - x is passed pre-transposed (xT [512, 4096], bf16) so the d_model
    contraction sits on the partition axis for both projection orientations.
  - q,k are produced transposed ([feat, row]) and RoPE'd in that layout
    (rotate_half done with a block-diag permutation matmul on the PE).
  - scores are computed transposed (S.T [keys, queries]) so softmax'd P
    feeds the av matmul as weights without any transpose.
  - causal mask applied by accumulating -240*max(0, k-q) into the scores
    PSUM via a rank-128 A.T@B matmul of constant triangular matrices.
  - exp on the scalar engine (PSUM -> bf16 SBUF), with 1/sqrt(hd) folded
    into the activation scale. No max-subtraction (scores are provably
    small for this problem's scale).
  - row sums for softmax come from an extra all-ones column appended to v.
  - weights/activations in bf16 (halves HBM traffic); PSUM accum is f32.
  - output partials DMA'd in bf16; the host reduces in f32.
  - xt double-buffered with explicit prefetch so the next batch/iteration's
    x reads overlap attention instead of waiting at the phase boundary.
"""

import sys
import math
from contextlib import ExitStack

sys.path.insert(0, "/opt/trn_rl_repo")

import numpy as np
import ml_dtypes

import concourse.bass as bass
import concourse.tile as tile
from concourse import bacc, mybir
from concourse.bass_utils import run_bass_kernel_spmd

F32 = mybir.dt.float32
F32R = mybir.dt.float32r
BF16 = mybir.dt.bfloat16
EXP = mybir.ActivationFunctionType.Exp

B = 2
T = 2048
D = 512
NH = 16
HD = 32
NCORES = 8
R = B * T            # 4096 rows, row = b*T + t
NHL = NH // NCORES   # 2 heads per core
KC = T // 128        # 16 key chunks per batch
SCALE = 1.0 / math.sqrt(HD)
MASK_VAL = -240.0

LOWP = True          # bf16 x / w_qkv / cos / sin / q / k path
XDT = BF16 if LOWP else F32R
NPX = ml_dtypes.bfloat16 if LOWP else np.float32


def _bcast_free(ap_2d, n_inner):
    """[P, n] -> [P, n, n_inner] AP with the inner dim broadcast (step 0)."""
    return bass.AP(
        tensor=ap_2d.tensor,
        offset=ap_2d.offset,
        ap=list(ap_2d.ap[:-1]) + [list(ap_2d.ap[-1]), [0, n_inner]],
    )


def _emit(tc, io, loop_k=1):
    nc = tc.nc
    with ExitStack() as ctx:
        cpool = ctx.enter_context(tc.tile_pool(name="consts", bufs=1))
        mpool = ctx.enter_context(tc.tile_pool(name="main", bufs=1))
        spool = ctx.enter_context(tc.tile_pool(name="small", bufs=3))
        ppool = ctx.enter_context(tc.tile_pool(name="pk", bufs=2))
        # single PSUM pool, tags shared across phases (8 banks total):
        #   tagA [128,1024] x2 = 4 banks  (qk-proj / scores)
        #   tagB [128,512]  x2 = 2 banks  (shift/vT/vtr/atps/outps)
        #   tagC [128,8,33] x2 = 2 banks  (av accumulator groups)
        psum = ctx.enter_context(tc.tile_pool(name="psum", bufs=1, space="PSUM"))

        def tile_a():
            return psum.tile([128, 1024], F32, tag="A", bufs=2, name="psA")

        def tile_b(p=128, w=512):
            return psum.tile([p, w], F32, tag="B", bufs=2, name="psB")

        def tile_c():
            return psum.tile([128, 8, HD + 1], F32, tag="C", bufs=2, name="psC")

        # ---- constants (one-time DMAs, spread over issue queues) ----
        wo = cpool.tile([64, 512], BF16, tag="wo")
        nc.scalar.dma_start(out=wo, in_=io["woT"])
        cmix = cpool.tile([128, 256], BF16, tag="cmix")
        nc.gpsimd.dma_start(out=cmix, in_=io["consts1"])
        trilA = cmix[:, 0:128]
        trilB = cmix[:, 128:256]
        identb = cpool.tile([128, 128], BF16, tag="identb")
        nc.gpsimd.dma_start(out=identb, in_=io["ident"])
        wqkv = []
        for dc in range(4):
            w_t = cpool.tile([128, 320], XDT, tag=f"wqkv{dc}")
            nc.gpsimd.dma_start(out=w_t, in_=io["wqkvT"][dc * 128:(dc + 1) * 128, :])
            wqkv.append(w_t)
        cosw = mpool.tile([128, T], XDT, tag="cosw")        # one batch (shared)
        sinw = mpool.tile([128, T], XDT, tag="sinw")
        nc.scalar.dma_start(out=cosw, in_=io["cosw"])
        nc.scalar.dma_start(out=sinw, in_=io["sinw"])

        # ---- persistent activations ----
        qkr = mpool.tile([128, R], XDT, tag="qkr")          # RoPE'd qT/kT
        ka = mpool.tile([64, R], XDT, tag="ka")             # k-half, base-aligned
        vall = mpool.tile([128, R // 128, NHL, HD + 1], BF16, tag="vall")
        ao = mpool.tile([128, B, KC, NHL, HD], BF16, tag="ao")  # attnout natural
        aoT = mpool.tile([64, R], BF16, tag="aoT")          # attnout transposed

        nc.vector.memset(vall[:, :, :, HD:HD + 1], 1.0)     # softmax-sum column

        def prefetch_xt(bb):
            xt = [mpool.tile([128, T], XDT, tag=f"xt{dc}", bufs=2,
                             name=f"xt{dc}") for dc in range(4)]
            for j in range(4):
                for dc in range(4):
                    nc.sync.dma_start(
                        out=xt[dc][:, j * 512:(j + 1) * 512],
                        in_=io["xT"][dc * 128:(dc + 1) * 128,
                                     bb * T + j * 512:bb * T + (j + 1) * 512],
                    )
            return xt

        def emit_proj(bb, xt):
            for jl in range(4):
                colb = slice(jl * 512, (jl + 1) * 512)          # batch-local
                cols = slice(bb * T + jl * 512, bb * T + (jl + 1) * 512)
                # qT/kT projection: [feat, row] = wqkT.T @ xT
                qk_ps = tile_b()
                for dc in range(4):
                    nc.tensor.matmul(
                        qk_ps, wqkv[dc][:, 0:128], xt[dc][:, colb],
                        start=(dc == 0), stop=(dc == 3),
                    )
                # rotated-half projection: extra weight columns hold the
                # feature-permuted q/k rows, so rotate_half needs no on-chip
                # permutation round-trip (PE stays dense)
                rot_ps = tile_b()
                for dc in range(4):
                    nc.tensor.matmul(
                        rot_ps, wqkv[dc][:, 128:256], xt[dc][:, colb],
                        start=(dc == 0), stop=(dc == 3),
                    )
                # qkr = qk*cos + rot*sin_signed, rounding to bf16 only at
                # the final add
                t1 = spool.tile([128, 512], F32, tag="t1")
                t2 = spool.tile([128, 512], F32, tag="t2")
                nc.vector.tensor_mul(t1, rot_ps, sinw[:, colb])
                nc.vector.tensor_mul(t2, qk_ps, cosw[:, colb])
                nc.vector.tensor_add(qkr[:, cols], t1, t2)
                # partition-aligned copy of the k rows (matmul requires lhsT
                # and rhs to share a base partition)
                nc.vector.tensor_copy(ka[:, cols], qkr[64:128, cols])

                # vT projection: [feat, row]
                vt_ps = tile_b(64)
                for dc in range(4):
                    nc.tensor.matmul(
                        vt_ps, wqkv[dc][:, 256:320], xt[dc][:, colb],
                        start=(dc == 0), stop=(dc == 3),
                    )
                vt_sb = spool.tile([64, 512], BF16, tag="vtsb")
                nc.scalar.copy(vt_sb, vt_ps)
                # transpose v back to natural [row, feat] (bf16 on copy-out);
                # 4 transposes share one PSUM bank (disjoint 64-col regions)
                vtr_ps = psum.tile([128, 256], BF16, tag="B", bufs=2,
                                   name="psBv")
                for jj in range(4):
                    nc.tensor.transpose(
                        vtr_ps[:, jj * 64:(jj + 1) * 64],
                        vt_sb[:, jj * 128:(jj + 1) * 128],
                        identb[0:64, 0:64],
                    )
                for jj in range(4):
                    nc.vector.tensor_copy(
                        vall[:, bb * KC + jl * 4 + jj, :, 0:HD],
                        vtr_ps[:, jj * 64:(jj + 1) * 64])

        def emit_attention(bb, hh):
            if True:
                qrow = 32 * hh            # q rows in qkr
                krow = 32 * hh            # k rows in ka
                ppks = []
                pavs = {}

                def av_column(qc):
                    # av column for qc (P rows kc<=qc all exist);
                    # 8 query chunks per PSUM bank, normalized per group
                    g = qc // 8
                    if qc % 8 == 0:
                        pavs[g] = tile_c()
                    slot = pavs[g][:, qc % 8, :]
                    for kp in range(qc + 1):
                        nc.tensor.matmul(
                            slot,
                            ppks[kp][:, 128 * (qc - kp):128 * (qc - kp) + 128],
                            vall[:, bb * KC + kp, hh, :],
                            start=(kp == 0), stop=(kp == qc),
                        )
                    if qc % 8 == 7:
                        # normalize this group: attnout = av / l
                        pav = pavs[g]
                        rl = spool.tile([128, 8, 1], F32, tag="rl")
                        nc.vector.reciprocal(rl, pav[:, :, HD:HD + 1])
                        nc.vector.tensor_mul(
                            ao[:, bb, g * 8:(g + 1) * 8, hh, :],
                            pav[:, :, 0:HD],
                            _bcast_free(rl[:, :, 0], HD),
                        )

                # av columns trail the score/exp stream by 2 key chunks so
                # the PE never stalls waiting for the exp it just queued
                for kc in range(KC + 2):
                    if kc < KC:
                        n_kc = T - 128 * kc
                        cw = 1024
                        kslc = slice(bb * T + 128 * kc, bb * T + 128 * (kc + 1))
                        # per-kc P tile: precise deps (av reads never block
                        # later exps) and half the packed-tile footprint
                        ppk = ppool.tile([128, n_kc], BF16, tag=f"ppk{kc}",
                                         bufs=(2 if kc < 5 else 1),
                                         name=f"ppk{kc}")
                        ppks.append(ppk)
                        for c0 in range(0, n_kc, cw):
                            nt = min(cw, n_kc - c0)
                            sc_ps = tile_a()
                            for c in range(c0, c0 + nt, 512):
                                ln = min(512, n_kc - c)
                                qslc = slice(bb * T + 128 * kc + c,
                                             bb * T + 128 * kc + c + ln)
                                nc.tensor.matmul(
                                    sc_ps[:, c - c0:c - c0 + ln],
                                    ka[krow:krow + 32, kslc],
                                    qkr[qrow:qrow + 32, qslc],
                                    start=True, stop=(c > 0),
                                    skip_group_check=True,
                                )
                            if c0 == 0:
                                # causal mask on the diagonal 128x128 block:
                                # accumulates -240*max(0, k-q)
                                nc.tensor.matmul(
                                    sc_ps[:, 0:128], trilA, trilB,
                                    start=False, stop=True,
                                    skip_group_check=True,
                                )
                            nc.scalar.activation(
                                out=ppk[:, c0:c0 + nt],
                                in_=sc_ps[:, 0:nt],
                                func=EXP, scale=SCALE,
                            )
                    if kc >= 2:
                        av_column(kc - 2)

        def emit_epilogue(bb, last):
            # transpose attnout group g, then immediately out-proj its 4
            # row chunks so the tail drains incrementally
            for g in range(4):
                at_ps = psum.tile([64, 512], BF16, tag="B", bufs=2,
                                  name="psBt")
                for jj in range(4):
                    qc = g * 4 + jj
                    nc.tensor.transpose(
                        at_ps[:, jj * 128:(jj + 1) * 128],
                        ao[:, bb, qc, :, :].rearrange("p a b -> p (a b)"),
                        identb,
                    )
                if last and g % 2 == 1:
                    nc.scalar.copy(
                        aoT[:, bb * T + g * 512:bb * T + (g + 1) * 512],
                        at_ps)
                else:
                    nc.vector.tensor_copy(
                        aoT[:, bb * T + g * 512:bb * T + (g + 1) * 512],
                        at_ps)
                for qc in range(g * 4, g * 4 + 4):
                    rc = bb * KC + qc
                    out_ps = tile_b()
                    nc.tensor.matmul(
                        out_ps, aoT[:, rc * 128:(rc + 1) * 128],
                        wo, start=True, stop=True,
                    )
                    out_sb = spool.tile([128, 512], BF16, tag="outsb", bufs=8)
                    nc.scalar.copy(out_sb, out_ps)
                    eng = nc.sync if qc % 2 == 0 else nc.gpsimd
                    eng.dma_start(
                        out=io["out_part"][rc * 128:(rc + 1) * 128, :],
                        in_=out_sb,
                    )

        # software-pipelined emission: later batches' proj and earlier
        # batches' epilogues fill engine gaps in the exp-paced attention;
        # xt prefetches overlap the next batch's x reads with attention
        xt0 = prefetch_xt(0)
        for _it in range(loop_k):
            xt1 = prefetch_xt(1)
            emit_proj(0, xt0)
            emit_attention(0, 0)
            xt0n = prefetch_xt(0) if _it < loop_k - 1 else None
            emit_attention(0, 1)
            emit_proj(1, xt1)
            emit_attention(1, 0)
            emit_epilogue(0, last=False)
            emit_attention(1, 1)
            emit_epilogue(1, last=True)
            xt0 = xt0n


def build_program(loop_k=1):
    nc = bacc.Bacc(
        "TRN2", target_bir_lowering=False, debug=False,
        enable_asserts=True, num_devices=NCORES,
    )
    io = {}
    for name, shape, dt_ in [
        ("xT", [D, R], XDT), ("wqkvT", [D, 320], XDT),
        ("woT", [64, D], BF16),
        ("cosw", [128, T], XDT), ("sinw", [128, T], XDT),
        ("consts1", [128, 256], BF16), ("ident", [128, 128], BF16),
    ]:
        io[name] = nc.dram_tensor(name, shape, dt_, kind="ExternalInput").ap()
    io["out_part"] = nc.dram_tensor("out_part", [R, D], BF16,
                                    kind="ExternalOutput").ap()
    with tile.TileContext(nc) as tc:
        _emit(tc, io, loop_k=loop_k)
    nc.compile()
    return nc


def host_constants():
    t = np.arange(T, dtype=np.float32)
    inv_freq = (1.0 / (10000.0 ** (np.arange(0, HD, 2, dtype=np.float32) / HD)))
    freqs = np.outer(t, inv_freq).astype(np.float32)      # (T, 16)
    emb = np.concatenate([freqs, freqs], axis=-1)         # (T, 32)
    cos = np.cos(emb).astype(np.float32)
    sin = np.sin(emb).astype(np.float32)
    cosw = np.tile(cos.T, (4, 1)).astype(NPX)             # (128, 2048)
    ssin = sin.T.copy()
    ssin[:HD // 2] *= -1.0                                # signed sin
    sinw = np.tile(ssin, (4, 1)).astype(NPX)

    a = np.arange(128)
    trilA = np.where(a[:, None] <= a[None, :], MASK_VAL, 0.0).astype(np.float32)
    trilB = np.where(a[:, None] > a[None, :], 1.0, 0.0).astype(np.float32)
    ident = np.eye(128, dtype=np.float32).astype(ml_dtypes.bfloat16)
    consts1 = np.concatenate([trilA, trilB], axis=1).astype(ml_dtypes.bfloat16)
    return dict(cosw=cosw, sinw=sinw, ident=ident,
                consts1=np.ascontiguousarray(consts1))


def core_inputs(x, w_qkv, w_o):
    """Per-core input maps (core c owns heads 2c, 2c+1)."""
    x = np.asarray(x, dtype=np.float32)
    w_qkv = np.asarray(w_qkv, dtype=np.float32)
    w_o = np.asarray(w_o, dtype=np.float32)
    xT = np.ascontiguousarray(x.reshape(R, D).T).astype(NPX)
    consts = host_constants()
    maps = []
    for c in range(NCORES):
        h0 = NHL * c
        qrows = w_qkv[h0 * HD:(h0 + NHL) * HD]                  # (64, 512)
        krows = w_qkv[D + h0 * HD:D + (h0 + NHL) * HD]
        vrows = w_qkv[2 * D + h0 * HD:2 * D + (h0 + NHL) * HD]
        qkrows = np.concatenate([qrows, krows], axis=0)        # (128, 512)
        blk, mm = np.divmod(np.arange(128), HD)
        rot_rows = qkrows[blk * HD + (mm + HD // 2) % HD]
        m = dict(consts)
        m["xT"] = xT
        m["wqkvT"] = np.ascontiguousarray(
            np.concatenate([qkrows, rot_rows, vrows], axis=0).T).astype(NPX)
        m["woT"] = np.ascontiguousarray(
            w_o[:, h0 * HD:(h0 + NHL) * HD].T).astype(ml_dtypes.bfloat16)
        maps.append(m)
    return maps


_PROG = None


def _get_prog():
    global _PROG
    if _PROG is None:
        _PROG = build_program()
    return _PROG


def kernel(x, w_qkv, w_o):
    nc = _get_prog()
    maps = core_inputs(x, w_qkv, w_o)
    res = run_bass_kernel_spmd(nc, maps, list(range(NCORES)))
    acc = np.zeros((R, D), dtype=np.float32)
    for i in range(NCORES):
        acc += res.results[i]["out_part"].astype(np.float32)
    return acc.reshape(B, T, D)


# revision 15
# speedup vs baseline: 1.7949x; 1.3786x over previous
"""Trainium2 Bass kernel for MultiHeadSelfAttention (RoPE + causal softmax).

Problem (hardcoded):
  x: (2, 2048, 512) f32, w_qkv: (1536, 512) f32, w_o: (512, 512) f32
  D_MODEL=512, N_HEADS=16, HEAD_DIM=32, ROPE_BASE=10000, causal.

Sharding: tensor-parallel over heads. Core c owns heads (2c, 2c+1) for both
batches. Each core computes its heads' q/k/v projections from the full x,
attention, and a Megatron-style row-parallel partial of the output
projection (out @ w_o.T restricted to its 64 input features). The host sums
the 8 bf16 partials in f32 (the row-parallel "unshard").

Design notes (HW-profiled on trn2; the kernel is bound by PE<->ACT coupling,
not raw engine throughput):
  - Everything is bf16 except PSUM accumulation: halves HBM traffic and
    keeps every matmul on the 1 cycle/row PE path (f32r scores measured
    ~1.4x slower than bf16 end to end).
  - x is passed pre-transposed (xT [512, 4096]) so d_model contracts on the
    partition axis; xt tiles are double-buffered and explicitly prefetched
    so the next batch/iteration's reads overlap attention.
  - rotate_half is folded into the projection: 128 extra weight columns
    hold the feature-permuted q/k rows, so RoPE needs no on-chip
    permutation round-trip; the DVE combines qk*cos + rot*sin straight
    from the two projection PSUMs.
  - scores are computed transposed (S.T [keys, queries]); softmax'd P
    feeds the av matmuls as PE weights without any transpose, and an extra
    all-ones v column yields the softmax row sums for free.
  - scores/exp pipeline: one [128,512] PSUM bank + one exp per 512-chunk,
    4-buf rotation, the two heads' chunks interleaved — the PE always has
    another chain's work while one chain's matmul->exp->reuse round-trip
    drains (this coupling, not ACT throughput, was the dominant cost).
  - causal mask: DVE multiplies the diagonal P block by an upper-triangle
    zero mask after exp (cheaper than PE mask matmuls and off the
    scores->exp critical path).
  - exp on the scalar engine (PSUM -> bf16 SBUF) with 1/sqrt(hd) folded
    into the activation scale; no max-subtraction (scores are provably
    small for this problem's scale).
  - epilogue (attnout transpose + out-proj) runs on the A-tag PSUM banks,
    which are idle after attention — sharing B with the next iteration's
    projection serialized the iteration boundary and cost ~16us.
"""

import sys
import math
from contextlib import ExitStack

sys.path.insert(0, "/opt/trn_rl_repo")

import numpy as np
import ml_dtypes

import concourse.bass as bass
import concourse.tile as tile
from concourse import bacc, mybir
from concourse.bass_utils import run_bass_kernel_spmd

F32 = mybir.dt.float32
F32R = mybir.dt.float32r
BF16 = mybir.dt.bfloat16
EXP = mybir.ActivationFunctionType.Exp

B = 2
T = 2048
D = 512
NH = 16
HD = 32
NCORES = 8
R = B * T            # 4096 rows, row = b*T + t
NHL = NH // NCORES   # 2 heads per core
KC = T // 128        # 16 key chunks per batch
SCALE = 1.0 / math.sqrt(HD)
MASK_VAL = -240.0

LOWP = True          # bf16 x / w_qkv / cos / sin / q / k path
XDT = BF16 if LOWP else F32R
NPX = ml_dtypes.bfloat16 if LOWP else np.float32


def _bcast_free(ap_2d, n_inner):
    """[P, n] -> [P, n, n_inner] AP with the inner dim broadcast (step 0)."""
    return bass.AP(
        tensor=ap_2d.tensor,
        offset=ap_2d.offset,
        ap=list(ap_2d.ap[:-1]) + [list(ap_2d.ap[-1]), [0, n_inner]],
    )


def _emit(tc, io, loop_k=1):
    nc = tc.nc
    with ExitStack() as ctx:
        cpool = ctx.enter_context(tc.tile_pool(name="consts", bufs=1))
        mpool = ctx.enter_context(tc.tile_pool(name="main", bufs=1))
        spool = ctx.enter_context(tc.tile_pool(name="small", bufs=3))
        ppool = ctx.enter_context(tc.tile_pool(name="pk", bufs=2))
        # single PSUM pool, tags shared across phases (8 banks total):
        #   tagA [128,512]  x4 = 4 banks  (scores / epilogue)
        #   tagB [128,512]  x2 = 2 banks  (proj qk/rot/vT/vtr)
        #   tagC [128,8,33] x2 = 2 banks  (av accumulator groups)
        psum = ctx.enter_context(tc.tile_pool(name="psum", bufs=1, space="PSUM"))

        def tile_a():
            return psum.tile([128, 512], F32, tag="A", bufs=4, name="psA")

        def tile_b(p=128, w=512):
            return psum.tile([p, w], F32, tag="B", bufs=2, name="psB")

        def tile_c():
            return psum.tile([128, 8, HD + 1], F32, tag="C", bufs=2, name="psC")

        # ---- constants (one-time DMAs, spread over issue queues) ----
        wo = cpool.tile([64, 512], BF16, tag="wo")
        nc.scalar.dma_start(out=wo, in_=io["woT"])
        ptri = cpool.tile([128, 128], BF16, tag="ptri")
        nc.gpsimd.dma_start(out=ptri, in_=io["consts1"])
        identb = cpool.tile([128, 128], BF16, tag="identb")
        nc.gpsimd.dma_start(out=identb, in_=io["ident"])
        wqkv = []
        for dc in range(4):
            w_t = cpool.tile([128, 320], XDT, tag=f"wqkv{dc}")
            nc.gpsimd.dma_start(out=w_t, in_=io["wqkvT"][dc * 128:(dc + 1) * 128, :])
            wqkv.append(w_t)
        cosw = mpool.tile([128, T], XDT, tag="cosw")        # one batch (shared)
        sinw = mpool.tile([128, T], XDT, tag="sinw")
        nc.scalar.dma_start(out=cosw, in_=io["cosw"])
        nc.scalar.dma_start(out=sinw, in_=io["sinw"])

        # ---- persistent activations ----
        qkr = mpool.tile([128, R], XDT, tag="qkr")          # RoPE'd qT/kT
        ka = mpool.tile([64, R], XDT, tag="ka")             # k-half, base-aligned
        vall = mpool.tile([128, R // 128, NHL, HD + 1], BF16, tag="vall")
        ao = mpool.tile([128, B, KC, NHL, HD], BF16, tag="ao")  # attnout natural
        aoT = mpool.tile([64, R], BF16, tag="aoT")          # attnout transposed

        nc.vector.memset(vall[:, :, :, HD:HD + 1], 1.0)     # softmax-sum column

        def prefetch_xt(bb):
            xt = [mpool.tile([128, T], XDT, tag=f"xt{dc}", bufs=2,
                             name=f"xt{dc}") for dc in range(4)]
            for j in range(4):
                for dc in range(4):
                    nc.sync.dma_start(
                        out=xt[dc][:, j * 512:(j + 1) * 512],
                        in_=io["xT"][dc * 128:(dc + 1) * 128,
                                     bb * T + j * 512:bb * T + (j + 1) * 512],
                    )
            return xt

        def emit_proj(bb, xt):
            for jl in range(4):
                colb = slice(jl * 512, (jl + 1) * 512)          # batch-local
                cols = slice(bb * T + jl * 512, bb * T + (jl + 1) * 512)
                # qT/kT projection: [feat, row] = wqkT.T @ xT
                qk_ps = tile_b()
                for dc in range(4):
                    nc.tensor.matmul(
                        qk_ps, wqkv[dc][:, 0:128], xt[dc][:, colb],
                        start=(dc == 0), stop=(dc == 3),
                    )
                # rotated-half projection: extra weight columns hold the
                # feature-permuted q/k rows, so rotate_half needs no on-chip
                # permutation round-trip (PE stays dense)
                rot_ps = tile_b()
                for dc in range(4):
                    nc.tensor.matmul(
                        rot_ps, wqkv[dc][:, 128:256], xt[dc][:, colb],
                        start=(dc == 0), stop=(dc == 3),
                    )
                # qkr = qk*cos + rot*sin_signed, rounding to bf16 only at
                # the final add
                t1 = spool.tile([128, 512], F32, tag="t1")
                t2 = spool.tile([128, 512], F32, tag="t2")
                nc.vector.tensor_mul(t1, rot_ps, sinw[:, colb])
                nc.vector.tensor_mul(t2, qk_ps, cosw[:, colb])
                nc.vector.tensor_add(qkr[:, cols], t1, t2)
                # partition-aligned copy of the k rows (matmul requires lhsT
                # and rhs to share a base partition)
                nc.vector.tensor_copy(ka[:, cols], qkr[64:128, cols])

                # vT projection: [feat, row]
                vt_ps = tile_b(64)
                for dc in range(4):
                    nc.tensor.matmul(
                        vt_ps, wqkv[dc][:, 256:320], xt[dc][:, colb],
                        start=(dc == 0), stop=(dc == 3),
                    )
                vt_sb = spool.tile([64, 512], BF16, tag="vtsb")
                nc.scalar.copy(vt_sb, vt_ps)
                # transpose v back to natural [row, feat] (bf16 on copy-out);
                # 4 transposes share one PSUM bank (disjoint 64-col regions)
                vtr_ps = psum.tile([128, 256], BF16, tag="B", bufs=2,
                                   name="psBv")
                for jj in range(4):
                    nc.tensor.transpose(
                        vtr_ps[:, jj * 64:(jj + 1) * 64],
                        vt_sb[:, jj * 128:(jj + 1) * 128],
                        identb[0:64, 0:64],
                    )
                for jj in range(4):
                    nc.vector.tensor_copy(
                        vall[:, bb * KC + jl * 4 + jj, :, 0:HD],
                        vtr_ps[:, jj * 64:(jj + 1) * 64])

        def emit_attention_pair(bb):
            # both heads interleaved at the kc level: while one head's exp
            # or av chain drains, the PE streams the other head's scores
            states = [dict(hh=hh, qrow=32 * hh, krow=32 * hh,
                           ppks=[], pavs={}) for hh in range(NHL)]

            def av_column(st, qc):
                # av column for qc (P rows kc<=qc all exist);
                # 8 query chunks per PSUM bank, normalized per group
                hh = st["hh"]
                g = qc // 8
                if qc % 8 == 0:
                    st["pavs"][g] = tile_c()
                slot = st["pavs"][g][:, qc % 8, :]
                for kp in range(qc + 1):
                    nc.tensor.matmul(
                        slot,
                        st["ppks"][kp][:, 128 * (qc - kp):128 * (qc - kp) + 128],
                        vall[:, bb * KC + kp, hh, :],
                        start=(kp == 0), stop=(kp == qc),
                    )
                if qc % 8 == 7:
                    # normalize this group: attnout = av / l
                    pav = st["pavs"][g]
                    rl = spool.tile([128, 8, 1], F32, tag="rl")
                    nc.vector.reciprocal(rl, pav[:, :, HD:HD + 1])
                    nc.vector.tensor_mul(
                        ao[:, bb, g * 8:(g + 1) * 8, hh, :],
                        pav[:, :, 0:HD],
                        _bcast_free(rl[:, :, 0], HD),
                    )

            def scores_chunk(st, kc, c):
                hh, qrow, krow = st["hh"], st["qrow"], st["krow"]
                n_kc = T - 128 * kc
                kslc = slice(bb * T + 128 * kc, bb * T + 128 * (kc + 1))
                ppk = st["ppks"][kc]
                ln = min(512, n_kc - c)
                qslc = slice(bb * T + 128 * kc + c,
                             bb * T + 128 * kc + c + ln)
                sc_ps = tile_a()
                nc.tensor.matmul(
                    sc_ps[:, 0:ln],
                    ka[krow:krow + 32, kslc],
                    qkr[qrow:qrow + 32, qslc],
                    start=True, stop=True,
                )
                nc.scalar.activation(
                    out=ppk[:, c:c + ln],
                    in_=sc_ps[:, 0:ln],
                    func=EXP, scale=SCALE,
                )
                if c == 0:
                    # causal mask: zero the upper triangle of the
                    # diagonal block's P (q < k) on the DVE; also
                    # zeroes those keys' softmax-sum contributions
                    nc.vector.tensor_mul(
                        ppk[:, 0:128], ppk[:, 0:128], ptri)

            # av columns trail the score/exp stream by 2 key chunks so
            # the PE never stalls waiting for the exp it just queued.
            # 512-chunks of the two heads alternate, so the 4-deep A-tile
            # rotation always has the other head's work to hide each
            # chain's matmul->exp->reuse latency
            for kc in range(KC + 2):
                if kc < KC:
                    n_kc = T - 128 * kc
                    for st in states:
                        st["ppks"].append(ppool.tile(
                            [128, n_kc], BF16, tag=f"ppk{kc}h{st['hh']}",
                            bufs=(2 if kc < 3 else 1),
                            name=f"ppk{kc}h{st['hh']}"))
                    for c in range(0, n_kc, 512):
                        for st in states:
                            scores_chunk(st, kc, c)
                for st in states:
                    if kc >= 2:
                        av_column(st, kc - 2)

        def emit_epilogue(bb, last):
            # transpose attnout group g, then immediately out-proj its 4
            # row chunks so the tail drains incrementally
            for g in range(4):
                at_ps = psum.tile([64, 512], BF16, tag="A", bufs=4,
                                  name="psAt")
                for jj in range(4):
                    qc = g * 4 + jj
                    nc.tensor.transpose(
                        at_ps[:, jj * 128:(jj + 1) * 128],
                        ao[:, bb, qc, :, :].rearrange("p a b -> p (a b)"),
                        identb,
                    )
                if last and g % 2 == 1:
                    nc.scalar.copy(
                        aoT[:, bb * T + g * 512:bb * T + (g + 1) * 512],
                        at_ps)
                else:
                    nc.vector.tensor_copy(
                        aoT[:, bb * T + g * 512:bb * T + (g + 1) * 512],
                        at_ps)
                for qc in range(g * 4, g * 4 + 4):
                    rc = bb * KC + qc
                    out_ps = psum.tile([128, 512], F32, tag="A", bufs=4,
                                       name="psAo")
                    nc.tensor.matmul(
                        out_ps, aoT[:, rc * 128:(rc + 1) * 128],
                        wo, start=True, stop=True,
                    )
                    out_sb = spool.tile([128, 512], BF16, tag="outsb", bufs=8)
                    nc.scalar.copy(out_sb, out_ps)
                    eng = nc.sync if qc % 2 == 0 else nc.gpsimd
                    eng.dma_start(
                        out=io["out_part"][rc * 128:(rc + 1) * 128, :],
                        in_=out_sb,
                    )

        # software-pipelined emission: later batches' proj and earlier
        # batches' epilogues fill engine gaps in the exp-paced attention;
        # xt prefetches overlap the next batch's x reads with attention
        xt0 = prefetch_xt(0)
        for _it in range(loop_k):
            xt1 = prefetch_xt(1)
            emit_proj(0, xt0)
            xt0n = prefetch_xt(0) if _it < loop_k - 1 else None
            emit_attention_pair(0)
            emit_proj(1, xt1)
            emit_attention_pair(1)
            emit_epilogue(0, last=False)
            emit_epilogue(1, last=True)
            xt0 = xt0n


def build_program(loop_k=1):
    nc = bacc.Bacc(
        "TRN2", target_bir_lowering=False, debug=False,
        enable_asserts=True, num_devices=NCORES,
    )
    io = {}
    for name, shape, dt_ in [
        ("xT", [D, R], XDT), ("wqkvT", [D, 320], XDT),
        ("woT", [64, D], BF16),
        ("cosw", [128, T], XDT), ("sinw", [128, T], XDT),
        ("consts1", [128, 128], BF16), ("ident", [128, 128], BF16),
    ]:
        io[name] = nc.dram_tensor(name, shape, dt_, kind="ExternalInput").ap()
    io["out_part"] = nc.dram_tensor("out_part", [R, D], BF16,
                                    kind="ExternalOutput").ap()
    with tile.TileContext(nc) as tc:
        _emit(tc, io, loop_k=loop_k)
    nc.compile()
    return nc


def host_constants():
    t = np.arange(T, dtype=np.float32)
    inv_freq = (1.0 / (10000.0 ** (np.arange(0, HD, 2, dtype=np.float32) / HD)))
    freqs = np.outer(t, inv_freq).astype(np.float32)      # (T, 16)
    emb = np.concatenate([freqs, freqs], axis=-1)         # (T, 32)
    cos = np.cos(emb).astype(np.float32)
    sin = np.sin(emb).astype(np.float32)
    cosw = np.tile(cos.T, (4, 1)).astype(NPX)             # (128, 2048)
    ssin = sin.T.copy()
    ssin[:HD // 2] *= -1.0                                # signed sin
    sinw = np.tile(ssin, (4, 1)).astype(NPX)

    a = np.arange(128)
    ident = np.eye(128, dtype=np.float32).astype(ml_dtypes.bfloat16)
    # keep P[k, q] only where q >= k (S.T layout: keys on partitions)
    consts1 = np.where(a[None, :] >= a[:, None], 1.0, 0.0
                       ).astype(ml_dtypes.bfloat16)
    return dict(cosw=cosw, sinw=sinw, ident=ident,
                consts1=np.ascontiguousarray(consts1))


def core_inputs(x, w_qkv, w_o):
    """Per-core input maps (core c owns heads 2c, 2c+1)."""
    x = np.asarray(x, dtype=np.float32)
    w_qkv = np.asarray(w_qkv, dtype=np.float32)
    w_o = np.asarray(w_o, dtype=np.float32)
    xT = np.ascontiguousarray(x.reshape(R, D).T).astype(NPX)
    consts = host_constants()
    maps = []
    for c in range(NCORES):
        h0 = NHL * c
        qrows = w_qkv[h0 * HD:(h0 + NHL) * HD]                  # (64, 512)
        krows = w_qkv[D + h0 * HD:D + (h0 + NHL) * HD]
        vrows = w_qkv[2 * D + h0 * HD:2 * D + (h0 + NHL) * HD]
        qkrows = np.concatenate([qrows, krows], axis=0)        # (128, 512)
        blk, mm = np.divmod(np.arange(128), HD)
        rot_rows = qkrows[blk * HD + (mm + HD // 2) % HD]
        m = dict(consts)
        m["xT"] = xT
        m["wqkvT"] = np.ascontiguousarray(
            np.concatenate([qkrows, rot_rows, vrows], axis=0).T).astype(NPX)
        m["woT"] = np.ascontiguousarray(
            w_o[:, h0 * HD:(h0 + NHL) * HD].T).astype(ml_dtypes.bfloat16)
        maps.append(m)
    return maps


_PROG = None


def _get_prog():
    global _PROG
    if _PROG is None:
        _PROG = build_program()
    return _PROG


def kernel(x, w_qkv, w_o):
    nc = _get_prog()
    maps = core_inputs(x, w_qkv, w_o)
    res = run_bass_kernel_spmd(nc, maps, list(range(NCORES)))
    acc = np.zeros((R, D), dtype=np.float32)
    for i in range(NCORES):
        acc += res.results[i]["out_part"].astype(np.float32)
    return acc.reshape(B, T, D)
